# revision 2
# baseline (speedup 1.0000x reference)
"""DeepSeek MoE gate routing kernel for Trainium2 (Bass/Tile), 8-core SPMD.

Problem: hidden_states [4, 4096, 4096] f32, gate weight [256, 4096] f32.
  logits = x @ W^T          (T=16384 tokens, E=256 experts, h=4096)
  scores = softmax(logits)
  topk_w, topk_i = top_k(scores, 8); topk_w = topk_w / sum(topk_w) * 2.5

Sharding: tokens split across 8 cores (2048 each); W replicated.

Per-core pipeline (16 tiles of 128 tokens), fp16 compute:
  - DMA x tile [128, 4096] f32 in natural layout (4 quarter-transfers,
    alternating the two HWDGE rings)
  - cast f32 -> fp16 on DVE/ACT (quarter granularity)
  - PE-transpose fp16 chunks [128t,128k] -> [128k,128t], 8 chunks into one
    PSUM bank [128, 1024] fp16 (1 cyc/row vs 2 for f32), copy PSUM->SBUF
  - matmul accumulate logits [128, 256] f32 over 32 k-chunks
    (stationary = x^T chunk fp16 w/ fast-weight-load, moving = W^T fp16)
  - W^T fp16 built on-chip once (cast + 64 PE transposes)
  - top-8: nc.vector.max (InstMax, descending top-8) + max_index
  - weights: exp(top8 - max) on ACT, sum/reciprocal/scale on DVE

fp16 quantization of x,W shifts ~0.3% of top-8 indices at near-ties
(weight l2 err ~3e-4), same regime as the f32r baseline (~0.16%).
"""

import numpy as np

import concourse.bass as bass
import concourse.mybir as mybir
from concourse import bacc
from concourse.bass_utils import run_bass_kernel_spmd
from concourse.masks import make_identity
from concourse.tile import TileContext

N_CORES = 8
H = 4096            # hidden size
E = 256             # n experts
TOPK = 8
T_FULL = 4 * 4096   # 16384 tokens
T_CORE = T_FULL // N_CORES  # 2048
P = 128             # partitions
N_TILES = T_CORE // P       # 16
KCH = H // P                # 32 contraction chunks
NB = 4              # transpose batches per tile (8 chunks each)
BCH = KCH // NB     # 8 chunks per batch -> [128, 1024] fp16 = one PSUM bank
Q = H // 4          # 1024-column DMA/cast quarters
SCALE = 2.5         # routed_scaling_factor

F32 = mybir.dt.float32
F16 = mybir.dt.float16


def build_bass():
    nc = bacc.Bacc(trn_type="TRN2")
    x = nc.dram_tensor("x", [T_CORE, H], F32, kind="ExternalInput")
    w = nc.dram_tensor("w", [E, H], F32, kind="ExternalInput")
    oid = nc.dram_tensor("oid", [T_CORE, TOPK], mybir.dt.int32, kind="ExternalOutput")
    owt = nc.dram_tensor("owt", [T_CORE, TOPK], F32, kind="ExternalOutput")

    with TileContext(nc) as tc:
        with (
            tc.tile_pool(name="const", bufs=1) as const_pool,
            tc.tile_pool(name="wnat", bufs=1) as wnat_pool,
            tc.tile_pool(name="w16", bufs=1) as w16_pool,
            tc.tile_pool(name="wt", bufs=1) as wt_pool,
            tc.tile_pool(name="xin", bufs=4) as x_pool,
            tc.tile_pool(name="x16", bufs=3) as x16_pool,
            tc.tile_pool(name="xt", bufs=8) as xt_pool,
            tc.tile_pool(name="pt", bufs=3, space="PSUM") as pt_pool,
            tc.tile_pool(name="pl", bufs=3, space="PSUM") as pl_pool,
            tc.tile_pool(name="small", bufs=2) as small_pool,
        ):
            ident = const_pool.tile([P, P], F16, tag="ident")
            make_identity(nc, ident)

            # x tile 0 DMA first so tile-0 cast/transposes start ASAP.
            xin0 = x_pool.tile([P, H], F32, tag="xin")
            for q in range(4):
                eng = nc.sync if q % 2 == 0 else nc.scalar
                eng.dma_start(out=xin0[:, q * Q:(q + 1) * Q], in_=x[0:P, q * Q:(q + 1) * Q])

            w0 = wnat_pool.tile([P, H], F32, tag="w0")
            w1 = wnat_pool.tile([P, H], F32, tag="w1")
            nc.sync.dma_start(out=w0[:, : H // 2], in_=w[0:P, : H // 2])
            nc.scalar.dma_start(out=w0[:, H // 2:], in_=w[0:P, H // 2:])
            nc.sync.dma_start(out=w1[:, : H // 2], in_=w[P:E, : H // 2])
            nc.scalar.dma_start(out=w1[:, H // 2:], in_=w[P:E, H // 2:])
            w_nat = (w0, w1)

            def cast_tile(xin, name):
                """f32 -> fp16 cast, quarter granularity, DVE/ACT alternating."""
                x16 = x16_pool.tile([P, H], F16, tag="x16", name=name)
                for q in range(4):
                    src = xin[:, q * Q:(q + 1) * Q]
                    dst = x16[:, q * Q:(q + 1) * Q]
                    if q % 2 == 0:
                        nc.vector.tensor_copy(dst, src)
                    else:
                        nc.scalar.copy(dst, src)
                return x16

            def transpose_batch(src16, b, copy_on_vector, name):
                """PE-transpose fp16 chunks 8b..8b+7 of src16 into one PSUM
                bank [128, 1024] fp16, copy to SBUF; returns the xT tile."""
                pt = pt_pool.tile([P, BCH * P], F16, tag="pt")
                for i in range(BCH):
                    c = BCH * b + i
                    nc.tensor.matmul(
                        pt[:, i * P:(i + 1) * P],
                        lhsT=src16[:, c * P:(c + 1) * P],
                        rhs=ident,
                        is_transpose=True,
                        start=(i == 0),
                        stop=(i == BCH - 1),
                    )
                xT = xt_pool.tile([P, BCH * P], F16, tag="xt", name=name)
                if copy_on_vector:
                    nc.vector.tensor_copy(xT, pt)
                else:
                    nc.scalar.copy(xT, pt)
                return xT

            # tile-0 cast + transposes first: they only need x0 (2 MiB) while
            # the W^T build below waits on the 4 MiB weight load.
            x16_0 = cast_tile(xin0, "x16_0")
            t0_xT = [
                transpose_batch(x16_0, b, b != 1, f"xT0_{b}") for b in range(NB)
            ]

            # ---- one-time: cast W to fp16 and build W^T [h, e] in SBUF as
            # 32 chunks [128, 256] fp16.  e-major order: all expert-half-0
            # batches first (need only w0, which lands before w1).
            w16 = []
            for e in range(2):
                wt16 = w16_pool.tile([P, H], F16, tag=f"w16_{e}")
                nc.vector.tensor_copy(wt16[:, : H // 2], w_nat[e][:, : H // 2])
                nc.scalar.copy(wt16[:, H // 2:], w_nat[e][:, H // 2:])
                w16.append(wt16)

            wT = wt_pool.tile([P, KCH * E], F16, tag="wt")
            wT_r = wT.rearrange("p (c eh) -> p c eh", eh=E)
            for e in range(2):
                for b in range(NB):  # 4 batches of 8 chunks each
                    pt = pt_pool.tile([P, BCH * P], F16, tag="pt")
                    for i in range(BCH):
                        c = BCH * b + i
                        nc.tensor.matmul(
                            pt[:, i * P:(i + 1) * P],
                            lhsT=w16[e][:, c * P:(c + 1) * P],
                            rhs=ident,
                            is_transpose=True,
                            start=(i == 0),
                            stop=(i == BCH - 1),
                        )
                    dst = wT_r[:, BCH * b:BCH * b + BCH, e * P:(e + 1) * P]
                    src = pt.rearrange("p (c q) -> p c q", q=P)
                    if b % 2 == 0:
                        nc.vector.tensor_copy(dst, src)
                    else:
                        nc.scalar.copy(dst, src)

            # ---- main loop over 16 token tiles ----
            for t in range(N_TILES):
                if t == 0:
                    t_xT = t0_xT
                else:
                    xin = x_pool.tile([P, H], F32, tag="xin")
                    for q in range(4):
                        eng = nc.sync if q % 2 == 0 else nc.scalar
                        eng.dma_start(
                            out=xin[:, q * Q:(q + 1) * Q],
                            in_=x[t * P:(t + 1) * P, q * Q:(q + 1) * Q],
                        )
                    x16 = cast_tile(xin, f"x16_{t}")
                    t_xT = [
                        transpose_batch(x16, b, b != 1, f"xT{t}_{b}")
                        for b in range(NB)
                    ]

                logits_ps = pl_pool.tile([P, E], F32, tag="logits")
                for b in range(NB):
                    xT = t_xT[b]
                    for i in range(BCH):
                        c = BCH * b + i
                        nc.tensor.matmul(
                            logits_ps,
                            lhsT=xT[:, i * P:(i + 1) * P],
                            rhs=wT[:, c * E:(c + 1) * E],
                            start=(c == 0),
                            stop=(c == KCH - 1),
                        )

                # ---- top-8 + softmax-normalized weights (straight off PSUM) ----
                mx = small_pool.tile([P, TOPK], F32, tag="mx")
                nc.vector.max(out=mx, in_=logits_ps)
                idx = small_pool.tile([P, TOPK], mybir.dt.uint32, tag="idx")
                nc.vector.max_index(out=idx, in_max=mx, in_values=logits_ps)
                negm = small_pool.tile([P, 1], F32, tag="negm")
                nc.vector.tensor_scalar_mul(negm, mx[:, 0:1], -1.0)
                e8 = small_pool.tile([P, TOPK], F32, tag="e8")
                nc.scalar.activation(
                    e8, mx, mybir.ActivationFunctionType.Exp, bias=negm, scale=1.0
                )
                s8 = small_pool.tile([P, 1], F32, tag="s8")
                nc.vector.reduce_sum(s8, e8, axis=mybir.AxisListType.X)
                rcp = small_pool.tile([P, 1], F32, tag="rcp")
                nc.vector.reciprocal(rcp, s8)
                wt8 = small_pool.tile([P, TOPK], F32, tag="wt8")
                nc.vector.tensor_scalar(
                    wt8, e8, scalar1=rcp, scalar2=SCALE,
                    op0=mybir.AluOpType.mult, op1=mybir.AluOpType.mult,
                )
                nc.scalar.dma_start(
                    out=oid[t * P:(t + 1) * P, :], in_=idx.bitcast(mybir.dt.int32)
                )
                nc.scalar.dma_start(out=owt[t * P:(t + 1) * P, :], in_=wt8)
    nc.compile()
    return nc


_NC_CACHE = {}


def _get_nc():
    if "nc" not in _NC_CACHE:
        _NC_CACHE["nc"] = build_bass()
    return _NC_CACHE["nc"]


def _ensure_ntff_hook():
    """This image's antenv lacks axon_hooks; shim it with the boot's own
    ctypes NTFF hook so trace=True works (only used by our test harness)."""
    import sys
    import types
    try:
        import antenv.axon_hooks  # noqa: F401
        return
    except ImportError:
        pass
    try:
        from trn_agent_boot.trn_boot import _ntff_profile_via_ctypes
        hook = _ntff_profile_via_ctypes("/opt/axon/libaxon_pjrt.so")
    except Exception:
        hook = None
    mod = types.ModuleType("antenv.axon_hooks")
    mod.get_axon_ntff_profile_hook = lambda: hook
    mod.set_axon_ntff_profile_hook = lambda h: None
    sys.modules["antenv.axon_hooks"] = mod
    import antenv
    antenv.axon_hooks = mod


def run(hidden_states, weight, trace=False):
    """Run on 8 NeuronCores; returns (topk_idx int32 [T,8], topk_w f32 [T,8], results)."""
    if trace:
        _ensure_ntff_hook()
    x = np.ascontiguousarray(
        np.asarray(hidden_states, dtype=np.float32).reshape(-1, H)
    )
    w = np.ascontiguousarray(np.asarray(weight, dtype=np.float32))
    assert x.shape == (T_FULL, H) and w.shape == (E, H)
    nc = _get_nc()
    in_maps = [
        {"x": np.ascontiguousarray(x[i * T_CORE:(i + 1) * T_CORE]), "w": w}
        for i in range(N_CORES)
    ]
    res = run_bass_kernel_spmd(
        nc, in_maps, core_ids=list(range(N_CORES)), trace=trace
    )
    idx = np.concatenate([r["oid"] for r in res.results], axis=0).astype(np.int32)
    wts = np.concatenate([r["owt"] for r in res.results], axis=0).astype(np.float32)
    return idx, wts, res


def kernel(hidden_states, weight):
    idx, wts, _ = run(hidden_states, weight)
    return idx, wts
